# revision 2
# baseline (speedup 1.0000x reference)
"""Trainium2 Bass kernel for nn_BatchASTEncoder (batched AST / complete-binary-tree
GNN message passing).

Math (per batch column b):
    h[p] = W_c @ encodes[node_ids[p, b]] + b_c                    (all 1023 tree positions)
    for level d = 8..0:  h[parent] += W_sum @ (h[left] + h[right]) + 2*b_sum
    node_list = relu(h[POSTORDER]);  out2 = max_p node_list

Distribution: data-parallel over the batch axis B=64 across 8 NeuronCores
(8 batch columns per core); encodes and the tiny weights are replicated.

Per-core layout: h is kept feature-major in SBUF as two [128, 8192] f32 tiles
(feature chunks 0:128 / 128:256, column r = b*1023 + p, padded to 8192).
Feature-major makes every tree-level pair-sum a strided free-dim vector op and
every matmul layout-preserving; the gather (row-major by nature) is rotated in
with PE transposes, and the output is rotated back the same way.
"""

import numpy as np

DEPTH = 10
P = 2**DEPTH - 1          # 1023
B = 64                    # full batch
E = 256
N_TOTAL = B * P           # 65472
N_CORES = 8
B_LOC = B // N_CORES      # 8
R = B_LOC * P             # 8184 columns per core
RP = 8192                 # padded to 64 blocks of 128
NBLK = RP // 128          # 64


def _postorder(p, out):
    if p >= P:
        return
    _postorder(2 * p + 1, out)
    _postorder(2 * p + 2, out)
    out.append(p)


_PO = []
_postorder(0, _PO)
POSTORDER = np.array(_PO, dtype=np.int32)

_NC_CACHE = [None]
LAST_RESULT = [None]      # BassKernelResults stash for test harnesses
TRACE = [False]


def _leaf_runs(blk):
    """Column runs [c0, c1, is_leaf) within block blk (local 0..128)."""
    runs = []
    cur = None
    start = 0
    for c in range(128):
        r = blk * 128 + c
        if r >= R:
            leaf = cur if cur is not None else True
        else:
            b, p = divmod(r, P)
            leaf = p >= 511
        if cur is None:
            cur, start = leaf, c
        elif leaf != cur:
            runs.append((start, c, cur))
            cur, start = leaf, c
    runs.append((start, 128, cur))
    return runs


def _build_nc():
    import concourse.bacc as bacc
    import concourse.mybir as mybir
    import concourse.tile as tile
    from concourse import bass
    from concourse.masks import make_identity

    f32 = mybir.dt.float32
    i32 = mybir.dt.int32
    AF = mybir.ActivationFunctionType
    AX = mybir.AxisListType

    nc = bacc.Bacc("TRN2", target_bir_lowering=False, debug=False)

    encodes = nc.dram_tensor("encodes", [N_TOTAL, E], f32, kind="ExternalInput")
    idx_d = nc.dram_tensor("idx", [128, NBLK], i32, kind="ExternalInput")
    wc_d = nc.dram_tensor("wc", [E, E], f32, kind="ExternalInput")
    ws_d = nc.dram_tensor("ws", [E, E], f32, kind="ExternalInput")
    bias_d = nc.dram_tensor("biases", [128, 4], f32, kind="ExternalInput")
    out_nl = nc.dram_tensor("out_nl", [RP, E], f32, kind="ExternalOutput")
    out_max = nc.dram_tensor("out_max", [128, 2 * B_LOC], f32, kind="ExternalOutput")

    with tile.TileContext(nc) as tc:
        with (
            tc.tile_pool(name="const", bufs=1) as cpool,
            tc.tile_pool(name="h", bufs=1) as hpool,
        ):
            wc0 = cpool.tile([128, E], f32)
            wc1 = cpool.tile([128, E], f32)
            ws0 = cpool.tile([128, E], f32)
            ws1 = cpool.tile([128, E], f32)
            bias = cpool.tile([128, 4], f32)
            idx = cpool.tile([128, NBLK], i32)
            ident = cpool.tile([128, 128], f32)
            nc.sync.dma_start(out=wc0[:], in_=wc_d[0:128, :])
            nc.sync.dma_start(out=wc1[:], in_=wc_d[128:256, :])
            nc.sync.dma_start(out=ws0[:], in_=ws_d[0:128, :])
            nc.sync.dma_start(out=ws1[:], in_=ws_d[128:256, :])
            nc.sync.dma_start(out=bias[:], in_=bias_d[:, :])
            nc.sync.dma_start(out=idx[:], in_=idx_d[:, :])
            make_identity(nc, ident[:])

            h0 = hpool.tile([128, RP], f32)
            h1 = hpool.tile([128, RP], f32)
            hs = (h0, h1)
            wcs = (wc0, wc1)
            wss = (ws0, ws1)

            # ---- Phase B: gather + W_c + transpose-in --------------------
            with (
                tc.tile_pool(name="g", bufs=3) as gpool,
                tc.tile_pool(name="gt", bufs=3) as gtpool,
                tc.tile_pool(name="tp", bufs=2, space="PSUM") as tppool,
                tc.tile_pool(name="hp", bufs=2, space="PSUM") as hppool,
            ):
                for blk in range(NBLK):
                    g = gpool.tile([128, E], f32)
                    nc.gpsimd.indirect_dma_start(
                        out=g[:],
                        out_offset=None,
                        in_=encodes[:, :],
                        in_offset=bass.IndirectOffsetOnAxis(
                            ap=idx[:, blk : blk + 1], axis=0
                        ),
                    )
                    gt = gtpool.tile([128, E], f32)
                    tp0 = tppool.tile([128, 128], f32, tag="tp0")
                    tp1 = tppool.tile([128, 128], f32, tag="tp1")
                    nc.tensor.transpose(out=tp0[:], in_=g[:, 0:128], identity=ident[:])
                    nc.tensor.transpose(out=tp1[:], in_=g[:, 128:256], identity=ident[:])
                    nc.vector.tensor_copy(out=gt[:, 0:128], in_=tp0[:])
                    nc.vector.tensor_copy(out=gt[:, 128:256], in_=tp1[:])
                    for e in range(2):
                        hp = hppool.tile([128, 128], f32, tag=f"hp{e}")
                        es = slice(128 * e, 128 * e + 128)
                        nc.tensor.matmul(
                            out=hp[:], lhsT=wc0[:, es], rhs=gt[:, 0:128],
                            start=True, stop=False,
                        )
                        nc.tensor.matmul(
                            out=hp[:], lhsT=wc1[:, es], rhs=gt[:, 128:256],
                            start=False, stop=True,
                        )
                        for c0, c1, leaf in _leaf_runs(blk):
                            bcol = e if leaf else 2 + e
                            nc.scalar.activation(
                                out=hs[e][:, blk * 128 + c0 : blk * 128 + c1],
                                in_=hp[:, c0:c1],
                                func=AF.Identity,
                                bias=bias[:, bcol : bcol + 1],
                                scale=1.0,
                            )

            # ---- Phase C: bottom-up tree ---------------------------------
            hr = [h[:, 0:R].rearrange("e (b q) -> e b q", b=B_LOC) for h in hs]
            with (
                tc.tile_pool(name="ks", bufs=1) as kspool,
                tc.tile_pool(name="cs", bufs=2, space="PSUM") as cspool,
            ):
                for d in range(DEPTH - 2, -1, -1):
                    p0 = 2**d - 1
                    n = 2**d
                    ks = [kspool.tile([128, B_LOC * n], f32, tag=f"ks{e}", name=f"ks{e}") for e in range(2)]
                    for e in range(2):
                        kids = hr[e][:, :, 2 * p0 + 1 : 2 * p0 + 1 + 2 * n]
                        kid2 = kids.rearrange("e b (n two) -> e b n two", two=2)
                        nc.vector.tensor_add(
                            out=ks[e][:].rearrange("e (b n) -> e b n", b=B_LOC),
                            in0=kid2[:, :, :, 0],
                            in1=kid2[:, :, :, 1],
                        )
                    ncols = B_LOC * n
                    nchunk = (ncols + 511) // 512
                    csz = ncols // nchunk
                    for ci in range(nchunk):
                        cs0 = ci * csz
                        for e in range(2):
                            cs = cspool.tile([128, csz], f32, tag=f"cs{e}")
                            es = slice(128 * e, 128 * e + 128)
                            nc.tensor.matmul(
                                out=cs[:], lhsT=ws0[:, es],
                                rhs=ks[0][:, cs0 : cs0 + csz],
                                start=True, stop=False,
                            )
                            nc.tensor.matmul(
                                out=cs[:], lhsT=ws1[:, es],
                                rhs=ks[1][:, cs0 : cs0 + csz],
                                start=False, stop=True,
                            )
                            # parent columns for this chunk: b range of size kb
                            kb = csz // n
                            b0 = ci * kb
                            nc.vector.tensor_add(
                                out=hr[e][:, b0 : b0 + kb, p0 : p0 + n],
                                in0=hr[e][:, b0 : b0 + kb, p0 : p0 + n],
                                in1=cs[:].rearrange("e (b n) -> e b n", b=kb),
                            )

            # ---- Phase D: max, relu, transpose-out, store ----------------
            with (
                tc.tile_pool(name="mx", bufs=1) as mxpool,
                tc.tile_pool(name="ob", bufs=3) as obpool,
                tc.tile_pool(name="tb", bufs=2, space="PSUM") as tbpool,
            ):
                mx = mxpool.tile([128, 2 * B_LOC], f32)
                for e in range(2):
                    mxp = mxpool.tile([128, B_LOC], f32, tag=f"mxp{e}")
                    nc.vector.tensor_reduce(
                        out=mxp[:], in_=hr[e][:, :, :], axis=AX.X,
                        op=mybir.AluOpType.max,
                    )
                    nc.scalar.activation(
                        out=mx[:, e * B_LOC : (e + 1) * B_LOC], in_=mxp[:],
                        func=AF.Relu,
                    )
                nc.sync.dma_start(out=out_max[:, :], in_=mx[:])

                for blk in range(NBLK):
                    ob = obpool.tile([128, E], f32)
                    for e in range(2):
                        tb = tbpool.tile([128, 128], f32, tag=f"tb{e}")
                        nc.tensor.transpose(
                            out=tb[:],
                            in_=hs[e][:, blk * 128 : (blk + 1) * 128],
                            identity=ident[:],
                        )
                        nc.scalar.activation(
                            out=ob[:, 128 * e : 128 * e + 128], in_=tb[:], func=AF.Relu
                        )
                    nc.sync.dma_start(
                        out=out_nl[blk * 128 : (blk + 1) * 128, :], in_=ob[:]
                    )

    nc.compile()
    return nc


def kernel(**inputs):
    from concourse.bass_utils import run_bass_kernel_spmd

    encodes = np.ascontiguousarray(np.asarray(inputs["encodes"], dtype=np.float32))
    node_ids = np.asarray(inputs["node_ids"])
    wc = np.ascontiguousarray(np.asarray(inputs["W_c_w"], dtype=np.float32).T)
    ws = np.ascontiguousarray(np.asarray(inputs["W_sum_w"], dtype=np.float32).T)
    bc = np.asarray(inputs["W_c_b"], dtype=np.float32)
    bs = np.asarray(inputs["W_sum_b"], dtype=np.float32)

    bias = np.zeros((128, 4), np.float32)
    bias[:, 0] = bc[0:128]
    bias[:, 1] = bc[128:256]
    bias[:, 2] = bc[0:128] + 2.0 * bs[0:128]
    bias[:, 3] = bc[128:256] + 2.0 * bs[128:256]

    in_maps = []
    for c in range(N_CORES):
        nid = np.asarray(node_ids[:, c * B_LOC : (c + 1) * B_LOC], dtype=np.int32)
        flat = np.zeros(RP, np.int32)
        flat[:R] = nid.T.reshape(-1)  # r = b*1023 + p
        idx = np.ascontiguousarray(flat.reshape(NBLK, 128).T)  # [part, blk]
        in_maps.append(
            {
                "encodes": encodes,
                "idx": idx,
                "wc": wc,
                "ws": ws,
                "biases": bias,
            }
        )

    if _NC_CACHE[0] is None:
        _NC_CACHE[0] = _build_nc()
    nc = _NC_CACHE[0]

    res = run_bass_kernel_spmd(
        nc, in_maps, core_ids=list(range(N_CORES)), trace=TRACE[0]
    )
    LAST_RESULT[0] = res

    node_list = np.empty((P, B, E), np.float32)
    mx = np.empty((B, E), np.float32)
    for c in range(N_CORES):
        r = res.results[c]
        nl = r["out_nl"][:R].reshape(B_LOC, P, E)  # [b_loc, p, e]
        node_list[:, c * B_LOC : (c + 1) * B_LOC, :] = nl.transpose(1, 0, 2)[POSTORDER]
        om = r["out_max"]  # [128, 16]
        mx[c * B_LOC : (c + 1) * B_LOC, 0:128] = om[:, 0:B_LOC].T
        mx[c * B_LOC : (c + 1) * B_LOC, 128:256] = om[:, B_LOC : 2 * B_LOC].T
    return node_list, mx
